# revision 1
# baseline (speedup 1.0000x reference)
"""Distributed flash-attention Bass kernel for 8 TRN2 NeuronCores.

Problem: nn_Attention (B=2, N=4096, C=512, H=8 heads, hd=64), f32 I/O.

Sharding (per the tensor-parallel hint): data parallel on batch x tensor
parallel on head PAIRS (2 batches x 4 pairs = 8 cores). Each core:
  - computes Q^T, K^T, V for ONLY its head pair but the FULL sequence
    (qkv weight columns sharded 4-way -> 4x less fill matmul work than
    replicating all heads per core),
  - runs flash attention for its 2 heads over all 4096x4096 scores
    (no-max softmax: scores provably within exp-safe range in bf16),
  - projects through its 128-row slice of w_proj, producing a PARTIAL
    [4096, 512] fp32 output (proj contracts only its own channels).
The host unshard step sums the 4 partial outputs per batch and adds
b_proj — the tensor-parallel output reduction done at gather time; no
device collectives needed.

Layouts (per core, SBUF, bf16 compute / f32 PSUM accumulation):
  X[cc]   [128, 4096]  x^T chunk (c-dims on partitions)
  W[cc]   [128, 384]   own qkv weight cols (q|k|v each 128) per c-chunk
  WP      [128, 512]   own w_proj^T rows
  Qp      [128, 4096]  q^T own pair (even head rows 0-63, odd 64-127)
  Kp      [128, 4096]  k^T own pair (same row split)
  VE[kt]  [128, 2, 65] v tile per 128 keys; col 64 of each head = 1.0
                       (ones column accumulates the softmax denominator)
  S^T = Kp_tile^T @ Qp -> PSUM [128 keys, 2x512] for both heads
  P = exp(scale*S): split across engines — ScalarE real exp on cols
      [0:768], DVE one-instruction Schraudolph-to-bf16 on [768:1024]
      (tensor_scalar mult+add with int16 output: the rounded affine
      result IS the bf16 bit pattern of exp; softmax's ratio structure
      cancels most of the ~3% elementwise ripple).
  o_ext[65, 512] += VE_tile^T @ P  (rows 0-63 = o^T, row 64 = sum_keys P)
  normalize: denominator row cast to bf16 (ScalarE), broadcast across
      partitions via a bf16 K=1 matmul, reciprocal_approx_fast + multiply
      on DVE, all off the PE critical path; SBUF->SBUF DMA into O^T.
  proj: out[128 n, 512] = O^T-chunk.T @ WP (single matmul — partial sum).

Scheduling: one fully-unrolled Tile graph; PV trails S/exp by one
key-tile across pass boundaries; fills and input DMAs are woven in with
a deadline-driven job queue; the final pass uses a quarter-pipelined
norm+proj tail. The ACT exp table load is triggered at t=0.
"""

import numpy as np
import ml_dtypes
from contextlib import ExitStack

import concourse.bass as bass
import concourse.mybir as mybir
import concourse.tile as tile
from concourse import bacc
from concourse.bass import ts, ds
from concourse.bass_utils import run_bass_kernel_spmd

BF16 = ml_dtypes.bfloat16
DT = mybir.dt.bfloat16
F32 = mybir.dt.float32
I16 = mybir.dt.int16
EXP = mybir.ActivationFunctionType.Exp

# Schraudolph exp-to-bf16 on DVE: bf16_bits(exp(x)) ~= rint(184.6627*x +
# 16256 + C). One tensor_scalar (mult, add) with int16 output.
SCH_A = 184.66267  # 128 * log2(e); multiplied by attn scale at use site
SCH_B = 16256.0 - 4.0
# Per-step split of the [128,1024] exp: ScalarE does cols [0:EXP_SPLIT],
# DVE does [EXP_SPLIT:1024].
EXP_SPLIT = 768

_LAST_RESULTS = None


def build_nc(N=4096, C=512, HD=64):
    """SPMD one-core graph: one batch, one head pair, full sequence."""
    KT = N // 128     # 128-key tiles
    KC = N // 512     # 512-key chunks
    QC = N // 512     # 512-query chunks (all queries now)
    NT = N // 128     # 128-row output tiles
    CC = C // 128     # 128-channel contraction chunks
    NTQ = NT // QC    # output tiles per query chunk
    scale = float(HD) ** -0.5

    nc = bacc.Bacc("TRN2", target_bir_lowering=False, debug=False)

    xt = nc.dram_tensor("xt", [C, N], DT, kind="ExternalInput").ap()
    wq = nc.dram_tensor("wq", [C, 384], DT, kind="ExternalInput").ap()
    wpt = nc.dram_tensor("wpt", [128, C], DT, kind="ExternalInput").ap()
    # bf16 partial outputs: halves the 8MB/core output DMA; the host
    # upcasts to f32 before summing the four partials per batch.
    out = nc.dram_tensor("out", [N, C], DT, kind="ExternalOutput").ap()

    with tile.TileContext(nc) as tc, ExitStack() as ctx:
        const = ctx.enter_context(tc.tile_pool(name="const", bufs=1))

        X = [const.tile([128, N], DT, tag=f"X{i}", name=f"X{i}") for i in range(CC)]
        W = [const.tile([128, 384], DT, tag=f"W{i}", name=f"W{i}") for i in range(CC)]
        WP = const.tile([128, C], DT, tag="WP")
        Qp = const.tile([128, N], DT, tag="Qp")
        Kp = const.tile([128, N], DT, tag="Kp")
        VE = [const.tile([128, 2, HD + 1], DT, tag=f"VE{i}", name=f"VE{i}") for i in range(KT)]
        OT = const.tile([128, N], DT, tag="OT")
        onesb = const.tile([128, 64], DT, tag="onesb")
        warm = const.tile([1, 8], F32, tag="warm")
        # Scratch tile for PE clock warm-up: deliberately never written, so
        # the matmuls reading it have no dependencies and run at t~0.
        wscr = const.tile([128, 256], DT, tag="wscr")

        # ---- input DMAs: attention-critical slices first; the x second
        # half is deadline-scheduled into the loop ----
        nc.vector.memset(onesb[:, :], 1.0)
        nc.vector.memset(wscr[:, :], 1.0)
        # Trigger the ~2.7us ACT exp-table load immediately so it overlaps
        # the input DMAs instead of delaying the first real exp.
        nc.scalar.activation(warm[:], onesb[0:1, 0:8], EXP)
        # Split the upfront DMAs at consumer granularity: the first q/k
        # fills need only W[:, 0:256] and X[:, 0:512]; a monolithic DMA
        # would stall them on data they don't need yet.
        for i in range(CC):
            nc.sync.dma_start(W[i][:, 0:256], wq[ts(i, 128), 0:256])
        for i in range(CC):
            nc.sync.dma_start(X[i][:, 0:512], xt[ts(i, 128), 0:512])
        for i in range(CC):
            nc.sync.dma_start(W[i][:, 256:384], wq[ts(i, 128), 256:384])
            nc.sync.dma_start(X[i][:, 512:1024], xt[ts(i, 128), 512:1024])
        for i in range(CC):
            nc.sync.dma_start(X[i][:, 1024:2048], xt[ts(i, 128), 1024:2048])

        with (
            tc.tile_pool(name="s_ps", bufs=2, space="PSUM") as s_ps,
            tc.tile_pool(name="o_ps", bufs=2, space="PSUM") as o_ps,
            tc.tile_pool(name="m_ps", bufs=2, space="PSUM") as m_ps,
            tc.tile_pool(name="p_sb", bufs=10) as p_sb,
            tc.tile_pool(name="t_sb", bufs=6) as t_sb,
            tc.tile_pool(name="ob_sb", bufs=3) as ob_sb,
        ):
            def emit_q_group(q2):
                ps = m_ps.tile([128, 512], F32, tag="m", name=f"qg{q2}")
                for cc in range(CC):
                    nc.tensor.matmul(
                        ps[:],
                        W[cc][:, 0:128],
                        X[cc][:, ts(q2, 512)],
                        start=(cc == 0),
                        stop=(cc == CC - 1),
                    )
                nc.vector.tensor_copy(Qp[:, ts(q2, 512)], ps[:])

            def emit_k_group(kc):
                ps = m_ps.tile([128, 512], F32, tag="m", name=f"kg{kc}")
                for cc in range(CC):
                    nc.tensor.matmul(
                        ps[:],
                        W[cc][:, 128:256],
                        X[cc][:, ts(kc, 512)],
                        start=(cc == 0),
                        stop=(cc == CC - 1),
                    )
                nc.vector.tensor_copy(Kp[:, ts(kc, 512)], ps[:])

            def emit_v_group(kt2):
                ps = m_ps.tile([128, 512], F32, tag="m", name=f"vg{kt2}")
                for cc in range(CC):
                    nc.tensor.matmul(
                        ps[:, 0:128],
                        X[cc][:, ts(kt2, 128)],
                        W[cc][:, 256:384],
                        start=(cc == 0),
                        stop=(cc == CC - 1),
                    )
                nc.vector.memset(VE[kt2][:, :, HD : HD + 1], 1.0)
                nc.vector.tensor_copy(
                    VE[kt2][:, :, 0:HD],
                    ps[:, 0:128].rearrange("p (h d) -> p h d", h=2),
                )

            # deferred (off the PE critical path) normalization + projection
            def make_norm(qc2, oc, half):
                def _n():
                    denb = t_sb.tile(
                        [65, 512], DT, tag="denb", name=f"dn{qc2}_{half}"
                    )
                    nc.scalar.copy(denb[64:65, :], oc[64:65, :])
                    rb = m_ps.tile([64, 512], F32, tag="m", name=f"rb{qc2}_{half}")
                    nc.tensor.matmul(
                        rb[:], onesb[64:65, :], denb[64:65, :], start=True, stop=True
                    )
                    rlb = t_sb.tile([64, 512], F32, tag="rlb", name=f"rl{qc2}_{half}")
                    nc.vector.reciprocal_approx_fast(rlb[:], rb[:])
                    tb = t_sb.tile([64, 512], DT, tag="tb", name=f"tb{qc2}_{half}")
                    nc.vector.tensor_mul(tb[:], oc[0:64, :], rlb[:])
                    nc.sync.dma_start(OT[ds(64 * half, 64), ts(qc2, 512)], tb[:])

                return _n

            def make_proj(nt):
                def _p():
                    pf = m_ps.tile([128, 512], F32, tag="m", name=f"pf{nt}")
                    nc.tensor.matmul(
                        pf[:], OT[:, ts(nt, 128)], WP[:], start=True, stop=True
                    )
                    ob = ob_sb.tile([128, C], DT, tag="ob", name=f"ob{nt}")
                    nc.vector.tensor_copy(ob[:], pf[:])
                    nc.sync.dma_start(out[ts(nt, 128), :], ob[:])

                return _p

            def pos_of(qc2, kt2):
                return qc2 * KT + kt2

            fill_jobs = []
            for k in range(KT):
                fill_jobs.append((max(0, k - 1), ("v", k)))
            for kc in range(2, KC):
                fill_jobs.append((max(0, 4 * kc - 2), ("k", kc)))
            for q2 in range(1, QC):
                fill_jobs.append((max(0, pos_of(q2, 0) - 16), ("q", q2)))

            def do_dma_job(job):
                i2, kind = job[1], job[2]
                if kind == "x2":
                    nc.sync.dma_start(
                        X[i2][:, N // 2 : N], xt[ts(i2, 128), N // 2 : N]
                    )
                else:
                    nc.sync.dma_start(WP[:], wpt[:, :])

            x2_dl = max(0, KT // 2 - 3)
            for i in range(CC):
                fill_jobs.append((x2_dl, ("d", i, "x2")))
            fill_jobs.append((KT - 8, ("d", 0, "wp")))
            fill_jobs.sort(key=lambda j: (j[0], j[1][0] != "d"))

            # -- PE clock (HAM) warm-up: ~3.4us of dummy matmuls on garbage
            # SBUF during the input-DMA wait, so the first real fills run at
            # 2.4 GHz instead of the cold 1.2 GHz default. Results land in a
            # scratch PSUM tile that is never read (start=True overwrites any
            # NaN garbage before reuse). --
            wps = m_ps.tile([128, 512], F32, tag="m", name="hamwarm")
            for i in range(16):
                nc.tensor.matmul(
                    wps[:, 0:256], wscr[:, 0:128], wscr[:, :],
                    start=True, stop=True,
                )
            # -- minimal upfront fill: just enough for the first S tiles --
            emit_q_group(0)
            for kc in range(2):
                emit_k_group(kc)

            def do_fill(job):
                kind = job[0]
                if kind == "v":
                    emit_v_group(job[1])
                elif kind == "k":
                    emit_k_group(job[1])
                elif kind == "q":
                    emit_q_group(job[1])
                else:
                    do_dma_job(job)

            DRAIN_AT = set(range(6, max(7, KT - 4), 4))

            pending = []  # deferred emissions, drained mid-pass

            held_projs = []  # pass-(QC-2) projs, deferred into the tail gap

            def make_tail(pqc, ocA, ocB):
                """Final pass: quarter-split the normalization so the DVE
                reciprocals pipeline with the per-tile output projections.
                The previous pass's held projections are emitted between the
                denominator broadcasts and the quarter loop: their inputs are
                long ready, so they fill the ~4us PE idle while the first
                quarter's recip/normalize/DMA chain resolves — instead of
                adding to the PE-saturated steady-state stream."""

                def _t():
                    rbs = []
                    for half, oc in ((0, ocA), (1, ocB)):
                        denb = t_sb.tile([65, 512], DT, tag="denb", name=f"dnt{half}")
                        # Halves on different engines so the two denominator
                        # casts run concurrently right after the last exp,
                        # shortening the tail's serial dependency chain.
                        if half == 0:
                            nc.scalar.copy(denb[64:65, :], oc[64:65, :])
                        else:
                            nc.vector.tensor_copy(denb[64:65, :], oc[64:65, :])
                        rb = s_ps.tile([64, 512], F32, tag="s", name=f"rbt{half}")
                        nc.tensor.matmul(
                            rb[:], onesb[64:65, :], denb[64:65, :],
                            start=True, stop=True,
                        )
                        rbs.append(rb)
                    for hp_ in held_projs:
                        hp_()
                    rlb = [
                        t_sb.tile([64, 512], F32, tag="rlb", name=f"rlbt{h}")
                        for h in range(2)
                    ]
                    tb = [
                        t_sb.tile([64, 512], DT, tag="tb", name=f"tbt{h}")
                        for h in range(2)
                    ]
                    for q in range(NTQ):
                        sl = ds(q * (512 // NTQ), 512 // NTQ)
                        for half, oc in ((0, ocA), (1, ocB)):
                            nc.vector.reciprocal_approx_fast(
                                rlb[half][:, sl], rbs[half][:, sl]
                            )
                            nc.vector.tensor_mul(
                                tb[half][:, sl], oc[0:64, sl], rlb[half][:, sl]
                            )
                            nc.sync.dma_start(
                                OT[
                                    ds(64 * half, 64),
                                    ds(pqc * 512 + q * (512 // NTQ), 512 // NTQ),
                                ],
                                tb[half][:, sl],
                            )
                        make_proj(pqc * NTQ + q)()

                return _t

            def finalize_pair(pqc, poA, poB):
                if pqc == QC - 1:
                    # Final pass: no bank-freeing copies needed (the o PSUM
                    # banks are never reused) — the tail reads the PV
                    # accumulators directly from PSUM.
                    pending.append(make_tail(pqc, poA, poB))
                    return
                ocA = t_sb.tile([128, 512], F32, tag="oc", name=f"ocA{pqc}")
                nc.vector.tensor_copy(ocA[0:65, :], poA[0:65, :])
                ocB = t_sb.tile([128, 512], F32, tag="oc", name=f"ocB{pqc}")
                nc.vector.tensor_copy(ocB[0:65, :], poB[0:65, :])
                pending.append(make_norm(pqc, ocA, 0))
                pending.append(make_norm(pqc, ocB, 1))
                if pqc == QC - 2:
                    held_projs.extend(make_proj(pqc * NTQ + i) for i in range(NTQ))
                else:
                    pending.extend(make_proj(pqc * NTQ + i) for i in range(NTQ))

            def emit_pv(pe):
                pp, pkt, poA, poB, pqc = pe
                last = pkt == KT - 1
                nc.tensor.matmul(
                    poA[0:65, :],
                    VE[pkt][:, 0, :],
                    pp[:, 0:512],
                    start=(pkt == 0),
                    stop=last,
                )
                nc.tensor.matmul(
                    poB[0:65, :],
                    VE[pkt][:, 1, :],
                    pp[:, 512:1024],
                    start=(pkt == 0),
                    stop=last,
                )
                if last:
                    finalize_pair(pqc, poA, poB)

            pend = None
            for qc in range(QC):
                cur = None
                for kt in range(KT):
                    pos = pos_of(qc, kt)
                    if kt == 0 and pend is not None:
                        # Emit the previous pass's trailing PV + finalize
                        # FIRST so its o-bank-freeing copies queue on the DVE
                        # ahead of this step's exp work — the new pass's
                        # first PV reuses those banks (o_ps WAR) and
                        # otherwise stalls ~300ns at every pass boundary.
                        emit_pv(pend)
                        pend = None
                    s = s_ps.tile([128, 1024], F32, tag="s", name=f"s{qc}_{kt}")
                    nc.tensor.matmul(
                        s[:, 0:512],
                        Kp[0:64, ts(kt, 128)],
                        Qp[0:64, ts(qc, 512)],
                        start=True,
                        stop=True,
                    )
                    nc.tensor.matmul(
                        s[:, 512:1024],
                        Kp[64:128, ts(kt, 128)],
                        Qp[64:128, ts(qc, 512)],
                        start=True,
                        stop=True,
                    )
                    p = p_sb.tile([128, 1024], DT, tag="p", name=f"p{qc}_{kt}")
                    # Exp split: ScalarE real exp on [0:ES], DVE Schraudolph
                    # on the rest. Pass 0 is fill-heavy on the DVE (qkv
                    # copies), so ScalarE takes it all there.
                    ES = 1024 if qc == 0 else EXP_SPLIT
                    nc.scalar.activation(p[:, 0:ES], s[:, 0:ES], EXP, scale=scale)
                    if ES < 1024:
                        nc.vector.tensor_scalar(
                            p[:, ES:1024].bitcast(I16),
                            s[:, ES:1024],
                            SCH_A * scale,
                            SCH_B,
                            mybir.AluOpType.mult,
                            mybir.AluOpType.add,
                        )
                    if pend is not None:
                        emit_pv(pend)
                    while fill_jobs and fill_jobs[0][0] <= pos:
                        do_fill(fill_jobs.pop(0)[1])
                    nfill = 1 if qc == 0 else (kt % 3 == 1)
                    while nfill > 0 and fill_jobs:
                        job = fill_jobs.pop(0)[1]
                        do_fill(job)
                        if job[0] != "d":
                            nfill -= 1
                    if kt in DRAIN_AT and pending:
                        pending.pop(0)()
                    if cur is None:
                        oA = o_ps.tile([128, 512], F32, tag="oext", name=f"oA{qc}")
                        oB = o_ps.tile([128, 512], F32, tag="oext", name=f"oB{qc}")
                        cur = (oA, oB)
                    pend = (p, kt, cur[0], cur[1], qc)
            emit_pv(pend)
            while pending:
                pending.pop(0)()

    nc.compile()
    return nc


_NC_CACHE = {}


def _get_nc(key=(4096, 512, 64)):
    if key not in _NC_CACHE:
        _NC_CACHE[key] = build_nc(*key)
    return _NC_CACHE[key]


def make_in_maps(x, w_qkv, w_proj, b_proj):
    C = x.shape[2]
    wqkvt = np.ascontiguousarray(w_qkv.T).astype(BF16)  # [C, 3C]
    wprojt = np.ascontiguousarray(w_proj.T).astype(BF16)  # [C_in, C_out]
    in_maps = []
    xtb = [np.ascontiguousarray(x[b].T).astype(BF16) for b in range(x.shape[0])]
    for c in range(8):
        b, hp = c // 4, c % 4
        sl = slice(128 * hp, 128 * (hp + 1))
        wq_own = np.ascontiguousarray(
            np.concatenate(
                [wqkvt[:, sl], wqkvt[:, C:][:, sl], wqkvt[:, 2 * C:][:, sl]],
                axis=1,
            )
        )
        wpt_own = np.ascontiguousarray(wprojt[sl, :])
        in_maps.append({"xt": xtb[b], "wq": wq_own, "wpt": wpt_own})
    return in_maps


def kernel(x, w_qkv, w_proj, b_proj):
    x = np.asarray(x, dtype=np.float32)
    w_qkv = np.asarray(w_qkv, dtype=np.float32)
    w_proj = np.asarray(w_proj, dtype=np.float32)
    b_proj = np.asarray(b_proj, dtype=np.float32)
    nc = _get_nc()
    in_maps = make_in_maps(x, w_qkv, w_proj, b_proj)
    res = run_bass_kernel_spmd(nc, in_maps, core_ids=list(range(8)))
    global _LAST_RESULTS
    _LAST_RESULTS = res
    B, N, C = x.shape
    out = np.empty((B, N, C), np.float32)
    for b in range(B):
        # tensor-parallel output reduction, done at gather/unshard time
        out[b] = (
            res.results[4 * b]["out"].astype(np.float32)
            + res.results[4 * b + 1]["out"].astype(np.float32)
            + res.results[4 * b + 2]["out"].astype(np.float32)
            + res.results[4 * b + 3]["out"].astype(np.float32)
            + b_proj
        )
    return out

